# revision 10
# baseline (speedup 1.0000x reference)
"""Trainium2 Bass kernel for sparse multi-headed attention.

Semantics (verified against the reference):
  q = x_q @ Wq.T + bq (per head, dk=32), same for k, v
  for each row s: attend to keys {s-c : c in (5,3,1,0), c <= s}
    score_c[s] = q[s].k[s-c] / sqrt(4)
    p = softmax over valid offsets
    attn[s] = sum_c p_c[s] * v[s-c]
  y = attn @ Wo.T + bo

Sharding: data-parallel over d_stock (8 stocks -> 8 cores). Each core
processes 4 (stock,batch) pairs = 2048 rows. Weights replicated.

Device layout: feature-major ("transposed") activations [256 feats, 2048
rows]; the host pre-transposes inputs during the shard step so no on-device
transposes are needed. Scores/softmax live in a pair-block layout
[128 partitions = 4 pair-blocks x (8 heads + 24 unused), 4 offsets x 512].
All matmul inputs are tf32 (float32r, pre-rounded on host); accumulation
and softmax stay fp32.
"""

import numpy as np

from concourse import bacc, bass, mybir, tile
from concourse.bass_utils import run_bass_kernel_spmd

DS, NB, S, DM, H, DK = 8, 4, 512, 256, 8, 32
CONS = (5, 3, 1, 0)
NCORES = 8
NPAIR = NB  # pairs per core (1 stock x 4 batches)
ROWS = NPAIR * S  # 2048
P = 128
PADC = 8  # zero pad columns in front of k/v for shifted reads
NEG = -1e9
SCALE = 0.5  # 1/sqrt(n_att)

f32 = mybir.dt.float32
f32r = mybir.dt.float32r
bf16 = mybir.dt.bfloat16
Act = mybir.ActivationFunctionType


def _emit(ctx, tc, nc, d, y_dram):
    dma_engs = [nc.sync, nc.scalar]
    dma_i = [0]

    def dma(out, in_):
        eng = dma_engs[dma_i[0] % 2]
        dma_i[0] += 1
        eng.dma_start(out=out, in_=in_)

    main = ctx.enter_context(tc.tile_pool(name="main", bufs=1))
    prodp = ctx.enter_context(tc.tile_pool(name="prodp", bufs=6))
    utmpp = ctx.enter_context(tc.tile_pool(name="utmpp", bufs=12))
    smx = ctx.enter_context(tc.tile_pool(name="smx", bufs=4))
    psum_pj = ctx.enter_context(tc.tile_pool(name="pj", bufs=2, space="PSUM"))
    psum_sc = ctx.enter_context(tc.tile_pool(name="sc", bufs=1, space="PSUM"))
    psum_y = ctx.enter_context(tc.tile_pool(name="yp", bufs=2, space="PSUM"))

    # ---------------- loads (q/k first; v deferred) ----------------
    xs = {}
    for name in ("xq", "xk", "xv"):
        for ch in range(2):
            dt = bf16 if name == "xv" else f32r
            xs[name, ch] = main.tile([P, ROWS], dt, name=f"{name}{ch}")
    ws = {}
    for name in ("wq", "wk", "wv", "wo"):
        for ch in range(2):
            dt = bf16 if name in ("wv", "wo") else f32r
            t = main.tile([P, DM], dt, name=f"{name}{ch}")
            if name not in ("wv", "wo"):
                dma(t[:], d[name][ch * P:(ch + 1) * P, :])
            ws[name, ch] = t
    # x loads split per 512-row slice so projections can start early
    for n in range(4):
        for ch in range(2):
            for name in ("xq", "xk"):
                dma(xs[name, ch][:, n * 512:(n + 1) * 512],
                    d[name][ch * P:(ch + 1) * P, n * 512:(n + 1) * 512])
    selkm = []
    bqkv = []
    for ch in range(2):
        t = main.tile([P, 224], f32r, name=f"selkm{ch}")
        dma(t[:], d["selkm"][ch])
        selkm.append(t)
        t = main.tile([P, 3], f32, name=f"bqkv{ch}")
        dma(t[:], d["bqkv"][ch * P:(ch + 1) * P, :])
        bqkv.append(t)

    # ---------------- q/k projections (PE, fp32r) ----------------
    qkv = {}
    for name in ("xq", "xk", "xv"):
        for ch in range(2):
            if name == "xq":
                qkv[name, ch] = (main.tile([P, ROWS], f32, name=f"pq{ch}"), 0)
            else:
                dt = bf16 if name == "xv" else f32
                big = main.tile([P, PADC + ROWS], dt, name=f"p{name[1]}{ch}")
                nc.vector.memset(big[:, 0:PADC], 0.0)
                qkv[name, ch] = (big, PADC)

    def project(name, wname, bcol, n, ch):
        big, off = qkv[name, ch]
        ps = psum_pj.tile([P, 512], f32, name="pjt", tag="pjt")
        for kch in range(2):
            nc.tensor.matmul(
                ps[:],
                lhsT=ws[wname, kch][:, ch * P:(ch + 1) * P],
                rhs=xs[name, kch][:, n * 512:(n + 1) * 512],
                start=(kch == 0), stop=(kch == 1))
        nc.scalar.activation(
            big[:, off + n * 512: off + (n + 1) * 512], ps[:],
            Act.Identity, bias=bqkv[ch][:, bcol:bcol + 1])

    for n in range(4):
        for ch in range(2):
            project("xq", "wq", 0, n, ch)
            project("xk", "wk", 1, n, ch)

    # deferred loads: v inputs, selectors for later phases
    for ch in range(2):
        dma(ws["wv", ch][:], d["wv"][ch * P:(ch + 1) * P, :])
        dma(ws["wo", ch][:], d["wo"][ch * P:(ch + 1) * P, :])
    for n in range(4):
        for ch in range(2):
            dma(xs["xv", ch][:, n * 512:(n + 1) * 512],
                d["xv"][ch * P:(ch + 1) * P, n * 512:(n + 1) * 512])
    selmk = []
    for p in range(NPAIR):
        row = []
        for ch in range(2):
            t = main.tile([P, P], bf16, name=f"selmk{p}{ch}")
            dma(t[:], d["selmk"][p, ch])
            row.append(t)
        selmk.append(row)
    ones1 = main.tile([1, P], bf16, name="ones1")
    dma(ones1[:], d["ones1"])
    bo_r = main.tile([1, DM], bf16, name="bo_r")
    dma(bo_r[:], d["bo"])

    # ---------------- scores ----------------
    # sc[32*pair + h, ci*512 + s] = q_h[s] . k_h[s-c] * 0.5
    sc = psum_sc.tile([P, 4 * 512], f32, name="scores")
    qb = {ch: qkv["xq", ch][0] for ch in range(2)}
    kb = {ch: qkv["xk", ch] for ch in range(2)}
    for ci, c in enumerate(CONS):
        out_ap = sc[:, ci * 512:(ci + 1) * 512]
        for p in range(NPAIR):
            for ch in range(2):
                pr = prodp.tile([P, 512], f32r, name="prod", tag="prod")
                k_t, koff = kb[ch]
                nc.vector.tensor_mul(
                    pr[:],
                    qb[ch][:, p * 512:(p + 1) * 512],
                    k_t[:, koff + p * 512 - c: koff + (p + 1) * 512 - c])
                nc.tensor.matmul(
                    out_ap,
                    lhsT=selkm[ch][:, 96 - 32 * p: 224 - 32 * p],
                    rhs=pr[:],
                    start=(p == 0 and ch == 0), stop=(p == 3 and ch == 1))
    # mask: scores for s_loc < c -> -1e9 (covers every pair block at once)
    for ci, c in enumerate(CONS):
        if c:
            nc.vector.memset(sc[:, ci * 512: ci * 512 + c], NEG)

    # ---------------- v projection (keeps PE busy during softmax) ----
    for n in range(4):
        for ch in range(2):
            project("xv", "wv", 2, n, ch)
    vb = {ch: qkv["xv", ch] for ch in range(2)}

    # ---------------- softmax over the 4 offsets (no max-sub: scores
    # are O(15) and masked lanes exp to 0) ----------------
    p_sb = main.tile([P, 4 * 512], bf16, name="p_sb")
    for ci in range(4):
        nc.scalar.activation(
            p_sb[:, ci * 512:(ci + 1) * 512], sc[:, ci * 512:(ci + 1) * 512],
            Act.Exp)
    d1 = smx.tile([P, 512], f32, name="d1", tag="smx")
    nc.vector.tensor_add(d1[:], p_sb[:, 0:512], p_sb[:, 512:1024])
    d2 = smx.tile([P, 512], f32, name="d2", tag="smx")
    nc.vector.tensor_add(d2[:], p_sb[:, 1024:1536], p_sb[:, 1536:2048])
    den = smx.tile([P, 512], f32, name="den", tag="smx")
    nc.vector.tensor_add(den[:], d1[:], d2[:])
    rcp = smx.tile([P, 512], f32, name="rcp", tag="smx")
    nc.vector.reciprocal(rcp[:], den[:])
    rcp_b = rcp[:].rearrange("p (o s) -> p o s", o=1).broadcast_to([P, 4, 512])
    p_v = p_sb[:].rearrange("p (c s) -> p c s", c=4)
    nc.vector.tensor_mul(p_v, p_v, rcp_b)

    # ---------------- attention + output projection, per pair --------
    y_view = y_dram.rearrange("(n p) d -> p n d", p=P)
    ybig = main.tile([P, 16 * DM], f32, name="ybig")
    for p in range(NPAIR):
        usum = {}
        for ch in range(2):
            v_t, voff = vb[ch]
            uts = []
            for ci, c in enumerate(CONS):
                bc = psum_pj.tile([P, 512], f32, name="bc", tag="pjt")
                nc.tensor.matmul(
                    bc[:],
                    lhsT=selmk[p][ch][:],
                    rhs=p_sb[:, ci * 512:(ci + 1) * 512],
                    start=True, stop=True)
                ut = utmpp.tile([P, 512], bf16, name="ut", tag="ut")
                nc.vector.tensor_mul(
                    ut[:], bc[:],
                    v_t[:, voff + p * 512 - c: voff + (p + 1) * 512 - c])
                uts.append(ut)
            s1 = utmpp.tile([P, 512], bf16, name="s1", tag="ut")
            nc.vector.tensor_add(s1[:], uts[0][:], uts[1][:])
            s2 = utmpp.tile([P, 512], bf16, name="s2", tag="ut")
            nc.vector.tensor_add(s2[:], uts[2][:], uts[3][:])
            us = utmpp.tile([P, 512], bf16, name="us", tag="ut")
            nc.vector.tensor_add(us[:], s1[:], s2[:])
            usum[ch] = us
        for tl in range(4):  # row-tiles within this pair
            t = 4 * p + tl
            yp = psum_y.tile([P, DM], f32, name="ypt", tag="ypt")
            for ch in range(2):
                nc.tensor.matmul(
                    yp[:],
                    lhsT=usum[ch][:, tl * P:(tl + 1) * P],
                    rhs=ws["wo", ch][:],
                    start=(ch == 0), stop=False)
            nc.tensor.matmul(
                yp[:], lhsT=ones1[:], rhs=bo_r[:], start=False, stop=True)
            nc.scalar.copy(ybig[:, t * DM:(t + 1) * DM], yp[:])
        nc.sync.dma_start(
            out=y_view[:, 4 * p:4 * p + 4, :],
            in_=ybig[:, 4 * p * DM:(4 * p + 4) * DM].rearrange(
                "p (n d) -> p n d", n=4))


def build_nc():
    from contextlib import ExitStack
    nc = bacc.Bacc(trn_type="TRN2", target_bir_lowering=False, debug=False)
    d = {}
    for name in ("xq", "xk", "xv"):
        dt = bf16 if name == "xv" else f32r
        d[name] = nc.dram_tensor(name, [DM, ROWS], dt, kind="ExternalInput").ap()
    for name in ("wq", "wk", "wv", "wo"):
        dt = bf16 if name in ("wv", "wo") else f32r
        d[name] = nc.dram_tensor(name, [DM, DM], dt, kind="ExternalInput").ap()
    d["bqkv"] = nc.dram_tensor("bqkv", [DM, 3], f32, kind="ExternalInput").ap()
    d["bo"] = nc.dram_tensor("bo", [1, DM], bf16, kind="ExternalInput").ap()
    d["ones1"] = nc.dram_tensor("ones1", [1, P], bf16, kind="ExternalInput").ap()
    d["selkm"] = nc.dram_tensor("selkm", [2, P, 224], f32r, kind="ExternalInput").ap()
    d["selmk"] = nc.dram_tensor("selmk", [NPAIR, 2, P, P], bf16, kind="ExternalInput").ap()
    y = nc.dram_tensor("y", [ROWS, DM], f32, kind="ExternalOutput").ap()
    with tile.TileContext(nc) as tc:
        with ExitStack() as ctx:
            _emit(ctx, tc, nc, d, y)
    nc.compile()
    return nc


def _round_tf32(a):
    """Round-to-nearest fp32 -> tf32 (10-bit mantissa)."""
    b = np.ascontiguousarray(a, dtype=np.float32).view(np.uint32)
    b = (b + np.uint32(0x1000)) & np.uint32(0xFFFFE000)
    return b.view(np.float32)


def make_shared_inputs(Wq, bq, Wk, bk, Wv, bv, Wo, bo):
    shared = {}
    shared["wq"] = _round_tf32(np.asarray(Wq, np.float32).T)
    shared["wk"] = _round_tf32(np.asarray(Wk, np.float32).T)
    import ml_dtypes
    shared["wv"] = np.ascontiguousarray(
        np.asarray(Wv, np.float32).T).astype(ml_dtypes.bfloat16)
    shared["wo"] = np.ascontiguousarray(
        np.asarray(Wo, np.float32).T).astype(ml_dtypes.bfloat16)
    shared["bqkv"] = np.ascontiguousarray(
        np.stack([bq, bk, bv], axis=1), dtype=np.float32)
    shared["bo"] = np.asarray(bo, np.float32).reshape(1, DM).astype(
        ml_dtypes.bfloat16)
    shared["ones1"] = np.ones((1, P), ml_dtypes.bfloat16)
    # selkm[ch, d, 96+h] = 0.5 iff h == global head of feature ch*128+d.
    # The score matmul for pair p uses lhsT = selkm[ch][:, 96-32p : 224-32p],
    # whose column j = 32p+h lands the head-h sum on psum partition 32p+h.
    selkm = np.zeros((2, P, 224), np.float32)
    for ch in range(2):
        for dd in range(P):
            selkm[ch, dd, 96 + ch * 4 + dd // 32] = SCALE
    shared["selkm"] = selkm
    # selmk[p, ch, 32p+j, d] = 1 iff global head of feature ch*128+d == j
    selmk = np.zeros((NPAIR, 2, P, P), ml_dtypes.bfloat16)
    for p in range(NPAIR):
        for ch in range(2):
            for dd in range(P):
                selmk[p, ch, 32 * p + ch * 4 + dd // 32, dd] = 1.0
    shared["selmk"] = selmk
    return shared


def make_core_inputs(query, key_in, value, core):
    # core i handles stock i: [4, 512, 256] -> feature-major [256, 2048]
    import ml_dtypes
    out = {}
    for name, x in (("xq", query), ("xk", key_in), ("xv", value)):
        xi = np.asarray(x[core], dtype=np.float32).reshape(ROWS, DM)
        if name == "xv":
            out[name] = np.ascontiguousarray(xi.T).astype(ml_dtypes.bfloat16)
        else:
            out[name] = _round_tf32(xi.T)
    return out


def kernel(query, key_in, value, Wq, bq, Wk, bk, Wv, bv, Wo, bo):
    nc = build_nc()
    shared = make_shared_inputs(Wq, bq, Wk, bk, Wv, bv, Wo, bo)
    in_maps = []
    for core in range(NCORES):
        m = dict(shared)
        m.update(make_core_inputs(query, key_in, value, core))
        in_maps.append(m)
    res = run_bass_kernel_spmd(nc, in_maps, list(range(NCORES))).results
    y = np.stack([res[i]["y"].reshape(NB, S, DM) for i in range(NCORES)])
    return y.astype(np.float32)
